# revision 13
# baseline (speedup 1.0000x reference)
"""Trainium2 Bass kernel for nn_AttnBlock: GroupNorm -> single-head spatial
self-attention (QKV 1x1 convs, softmax over 1024 positions, AV) -> proj 1x1
conv -> residual.

Sharding: data-parallel over batch. B=16 -> 2 batches per NeuronCore x 8.

v2 design (vs the fp32r v1 baseline at ~150us):
  * QK fusion: biases q_b,k_b are zero, so scores = h^T (Wk^T Wq) h. The
    fused Wm = Wk^T Wq is computed host-side; the q- and k-projections
    collapse into one "t = Wm h" projection (saves 1 of 4 C x C GEMMs and
    all k-side evacuations).
  * All heavy matmuls run fp8 e4m3 with MatmulPerfMode.DoubleRow (2 fp8
    weights per PE cell = 256-row virtual array, ~2x throughput). Operands
    are kept in "stacked-subtile" layout [128, n_sub, free] so a pair slice
    [:, 2i:2i+2, :] is directly the DoubleRow [Ki, Ko=2, dim] AP.
  * Weights are pre-scaled by 16 on the host so their ~N(0, 1/512) entries
    sit in fp8's normal range; the inverse scaling folds for free into the
    exp activation (scores path) and the proj-evacuation multiplier.
  * Softmax needs no max-subtraction (logits ~N(0,1)); a fixed -2.0 bias
    inside the exp keeps e^s under fp8's 240 max. The denominator is an
    all-ones DoubleRow matmul over the quantized probs, so normalization is
    self-consistent with the quantization.
  * rstd = exp(-0.5 * ln(var+eps)) keeps every ACT function (Square, Ln,
    Exp, Copy) inside one activation-table set -> no table reload churn.
  * DMA order: GN consts + batch-0 x first, then weights; output stores go
    out in [128,512] chunks on the idle GpSimd queue to shorten the tail.
"""

import os
import sys

import numpy as np

for _p in ("/opt/trn_rl_repo", "/root/.axon_site/_ro/trn_rl_repo"):
    if os.path.isdir(_p) and _p not in sys.path:
        sys.path.insert(0, _p)

import concourse.bacc as bacc
import concourse.tile as tile
import concourse.mybir as mybir
from concourse.alu_op_type import AluOpType
from concourse.bass_utils import run_bass_kernel_spmd

B, C, H, W = 16, 512, 32, 32
N = H * W                  # 1024 spatial positions
GROUPS = 32
GS = C // GROUPS           # 16 channels per group
NCORES = 8
BPC = B // NCORES          # batches per core
CT = C // 128              # channel partition-subtiles (4)
NT = N // 128              # position partition-subtiles (8)
NCH = N // 512             # 512-wide free chunks (2)
EPS = 1e-5
ATTN_SCALE = float(C) ** -0.5
WSCALE = 16.0              # host-side weight pre-scale into fp8 normal range
EXP_BIAS = -2.0            # keeps exp(logit) <= e^3.4 ~ 30 < 240 (fp8e4 max)

F32 = mybir.dt.float32
F8 = mybir.dt.float8e4
DR = mybir.MatmulPerfMode.DoubleRow
Act = mybir.ActivationFunctionType

LAST_RESULTS = None        # BassKernelResults of the most recent run (for test.py)

_PROGRAM_CACHE = {}


def _build_program(flags, loop_reps=None):
    """Build the per-core Bass program. flags = (qb_nz, vb_nz, pb_nz).

    loop_reps: if set, wrap the whole per-core body in a hardware For_i loop
    (benchmarking only -- output identical each rep since xs is re-read)."""
    qb_nz, vb_nz, pb_nz = flags
    nc = bacc.Bacc(
        "TRN2",
        target_bir_lowering=False,
        debug=False,
        enable_asserts=False,
        num_devices=NCORES,
    )

    def din(name, shape, dt=F32):
        return nc.dram_tensor(name, shape, dt, kind="ExternalInput").ap()

    xs = din("xs", [BPC, CT, 128, N])
    gmat_d = din("gmat", [128, 128])
    gnw_d = din("gnw", [128, CT])
    gnb_d = din("gnb", [128, CT])
    wm_d = din("wm", [128, CT, C], F8)     # fused (Wk^T Wq)^T, stacked subtiles
    wv_d = din("wv", [128, CT, C], F8)
    wp_d = din("wp", [128, CT, C], F8)
    ones_d = din("ones", [128, 2, 128], F8)
    uq_d = din("uq", [128, CT, 1], F8) if qb_nz else None
    vb_d = din("vb", [128, C]) if vb_nz else None
    pb_d = din("pb", [CT, 128, 1]) if pb_nz else None

    out_d = nc.dram_tensor("out", [BPC, CT, 128, N], F32, kind="ExternalOutput").ap()

    with tile.TileContext(nc) as tc:
        _emit(tc, xs, gmat_d, gnw_d, gnb_d, wm_d, wv_d, wp_d, ones_d,
              uq_d, vb_d, pb_d, out_d, loop_reps=loop_reps)
    nc.compile()
    return nc


def _emit(tc, xs, gmat_d, gnw_d, gnb_d, wm_d, wv_d, wp_d, ones_d,
          uq_d, vb_d, pb_d, out_d, loop_reps=None):
    nc = tc.nc
    from contextlib import ExitStack
    ctx = ExitStack()
    with ctx:
        consts = ctx.enter_context(tc.tile_pool(name="consts", bufs=1))
        xin = ctx.enter_context(tc.tile_pool(name="xin", bufs=2 * CT))
        scr = ctx.enter_context(tc.tile_pool(name="scr", bufs=2))
        small = ctx.enter_context(tc.tile_pool(name="small", bufs=16))
        h8p = ctx.enter_context(tc.tile_pool(name="h8p", bufs=2))
        t8p = ctx.enter_context(tc.tile_pool(name="t8p", bufs=2))
        v8p = ctx.enter_context(tc.tile_pool(name="v8p", bufs=2))
        a8p = ctx.enter_context(tc.tile_pool(name="a8p", bufs=2))
        h28p = ctx.enter_context(tc.tile_pool(name="h28p", bufs=2))
        rpool = ctx.enter_context(tc.tile_pool(name="rpool", bufs=4))
        psmain = ctx.enter_context(tc.tile_pool(name="psmain", bufs=6, space="PSUM"))
        psgn = ctx.enter_context(tc.tile_pool(name="psgn", bufs=2, space="PSUM"))

        # ---- constants: GN-related + batch-0 x go first in the DMA queue ----
        def load_const(tag, src, shape, dt=F32):
            t = consts.tile(shape, dt, tag=tag)
            nc.sync.dma_start(out=t, in_=src)
            return t

        gmat_sb = load_const("gmat", gmat_d, [128, 128])
        gnw_sb = load_const("gnw", gnw_d, [128, CT])
        gnb_sb = load_const("gnb", gnb_d, [128, CT])
        pb_sb = [load_const(f"pb{ci}", pb_d[ci], [128, 1]) for ci in range(CT)] if pb_d is not None else None

        def load_x(b):
            xt = []
            for t in range(CT):
                a = xin.tile([128, N], F32, tag="xt")
                nc.sync.dma_start(out=a, in_=xs[b, t])
                xt.append(a)
            return xt

        xts = {0: load_x(0)}

        wm_sb = load_const("wm", wm_d, [128, CT, C], F8)
        wv_sb = load_const("wv", wv_d, [128, CT, C], F8)
        ones_sb = load_const("ones", ones_d, [128, 2, 128], F8)
        wp_sb = load_const("wp", wp_d, [128, CT, C], F8)
        uq_sb = load_const("uq", uq_d, [128, CT, 1], F8) if uq_d is not None else None
        vb_sb = load_const("vb", vb_d, [128, C]) if vb_d is not None else None

        ebias_sb = consts.tile([128, 1], F32, tag="ebias")
        nc.vector.memset(ebias_sb, EXP_BIAS)
        magic_sb = consts.tile([128, CT], mybir.dt.uint32, tag="magic")
        nc.vector.memset(magic_sb, 0x5F3759DF)

        def group_norm(xt):
            """GN over one batch's 4 c-subtiles -> h8 [128, CT, N] fp8.

            Per-tile sums feed one gmat matmul (group-reduce + broadcast for
            all 4 subtiles at once); the scalar tail -- including rstd via
            Newton-iterated fast-inverse-sqrt on DVE -- runs vectorized on
            [128, CT] so ACT only ever needs Square/Copy/Exp (one table set).
            """
            h8 = h8p.tile([128, CT, N], F8, tag="h8")
            pst = small.tile([128, 2 * CT], F32, tag="pst")
            for t in range(CT):
                nc.vector.reduce_sum(pst[:, 2*t:2*t+1], xt[t], mybir.AxisListType.X)
                sq = scr.tile([128, N], F32, tag="scr")
                nc.scalar.activation(sq, xt[t], Act.Square,
                                     accum_out=pst[:, 2*t+1:2*t+2])
            gps = psgn.tile([128, 2 * CT], F32, tag="gn")
            nc.tensor.matmul(gps, lhsT=gmat_sb, rhs=pst, start=True, stop=True)
            st = small.tile([128, 2 * CT], F32, tag="gst")
            nc.vector.tensor_copy(out=st, in_=gps)
            mean, ex2 = st[:, 0::2], st[:, 1::2]
            va = small.tile([128, CT], F32, tag="va")
            nc.vector.tensor_tensor(va, mean, mean, AluOpType.mult)
            nc.vector.tensor_tensor(va, ex2, va, AluOpType.subtract)
            nc.vector.tensor_scalar_add(va, va, EPS)
            y = small.tile([128, CT], F32, tag="y")
            yu = y.bitcast(mybir.dt.uint32)
            nc.vector.tensor_scalar(yu, va.bitcast(mybir.dt.uint32), 1, None,
                                    op0=AluOpType.logical_shift_right)
            nc.vector.tensor_tensor(yu, magic_sb, yu, AluOpType.subtract)
            w = small.tile([128, CT], F32, tag="w")
            for _ in range(2):
                nc.vector.tensor_tensor(w, y, y, AluOpType.mult)
                nc.vector.tensor_tensor(w, w, va, AluOpType.mult)
                nc.vector.tensor_scalar(w, w, -0.5, 1.5,
                                        op0=AluOpType.mult, op1=AluOpType.add)
                nc.vector.tensor_tensor(y, y, w, AluOpType.mult)
            s = small.tile([128, CT], F32, tag="s")
            nc.vector.tensor_tensor(s, y, gnw_sb, AluOpType.mult)
            bp = small.tile([128, CT], F32, tag="bp")
            nc.vector.tensor_tensor(bp, mean, s, AluOpType.mult)
            nc.vector.tensor_tensor(bp, gnb_sb, bp, AluOpType.subtract)
            for t in range(CT):
                nc.vector.tensor_scalar(h8[:, t, :], xt[t], s[:, t:t+1], bp[:, t:t+1],
                                        op0=AluOpType.mult, op1=AluOpType.add)
            return h8

        def emit_tv(b, xt, h8):
            """t = (Wk^T Wq) h and vT projections for batch b."""
            t8 = t8p.tile([128, CT, N], F8, tag="t8")
            for dt_ in range(CT):
                dsl = slice(128 * dt_, 128 * (dt_ + 1))
                for nch in range(NCH):
                    nsl = slice(512 * nch, 512 * (nch + 1))
                    ps = psmain.tile([128, 512], F32, tag="ps")
                    for i in range(CT // 2):
                        nc.tensor.matmul(ps, lhsT=wm_sb[:, 2*i:2*i+2, dsl],
                                         rhs=h8[:, 2*i:2*i+2, nsl],
                                         start=(i == 0), stop=(i == CT // 2 - 1),
                                         perf_mode=DR)
                    nc.scalar.copy(out=t8[:, dt_, nsl], in_=ps)

            # ---- vT[key_sub, c] via h as stationary ----
            v8 = v8p.tile([128, NT, C], F8, tag="v8")
            for nt_ in range(NT):
                psl = slice(128 * nt_, 128 * (nt_ + 1))
                ps = psmain.tile([128, 512], F32, tag="ps")
                for i in range(CT // 2):
                    nc.tensor.matmul(ps, lhsT=h8[:, 2*i:2*i+2, psl],
                                     rhs=wv_sb[:, 2*i:2*i+2, :],
                                     start=(i == 0), stop=(i == CT // 2 - 1),
                                     perf_mode=DR)
                if vb_sb is not None:
                    nc.vector.tensor_tensor(v8[:, nt_, :], ps, vb_sb, AluOpType.add)
                else:
                    nc.vector.tensor_copy(out=v8[:, nt_, :], in_=ps)
            return t8, v8

        def emit_attn(b, xt, h8, t8, v8):
            # ---- scoresT + exp: a8[key_sub, q] fp8 probs ----
            # psum = 16*s_raw; logits = s_raw*C^-0.5; exp(logits - 2) via
            # activation scale+bias, output quantized to fp8
            a8 = a8p.tile([128, NT, N], F8, tag="a8")
            if uq_sb is not None:
                sbias = {}
                for kt in range(NT):
                    ksl = slice(128 * kt, 128 * (kt + 1))
                    psb = psgn.tile([128, 1], F32, tag="gn")
                    for i in range(CT):
                        nc.tensor.matmul(psb, lhsT=h8[:, i, ksl], rhs=uq_sb[:, i, :],
                                         start=(i == 0), stop=(i == CT - 1))
                    bt = small.tile([128, 1], F32, tag="bt")
                    nc.vector.tensor_scalar(bt, psb, ATTN_SCALE / WSCALE, EXP_BIAS,
                                            op0=AluOpType.mult, op1=AluOpType.add)
                    sbias[kt] = bt
            for kt in range(NT):
                ksl = slice(128 * kt, 128 * (kt + 1))
                for nch in range(NCH):
                    qsl = slice(512 * nch, 512 * (nch + 1))
                    ps = psmain.tile([128, 512], F32, tag="ps")
                    for i in range(CT // 2):
                        nc.tensor.matmul(ps, lhsT=h8[:, 2*i:2*i+2, ksl],
                                         rhs=t8[:, 2*i:2*i+2, qsl],
                                         start=(i == 0), stop=(i == CT // 2 - 1),
                                         perf_mode=DR)
                    bias_arg = sbias[kt] if uq_sb is not None else ebias_sb
                    nc.scalar.activation(a8[:, kt, qsl], ps, Act.Exp,
                                         bias=bias_arg,
                                         scale=ATTN_SCALE / WSCALE)

            # ---- softmax denominator from the quantized probs ----
            recips = []
            for nch in range(NCH):
                qsl = slice(512 * nch, 512 * (nch + 1))
                ps = psmain.tile([128, 512], F32, tag="ps")
                for i in range(NT // 2):
                    nc.tensor.matmul(ps, lhsT=ones_sb,
                                     rhs=a8[:, 2*i:2*i+2, qsl],
                                     start=(i == 0), stop=(i == NT // 2 - 1),
                                     perf_mode=DR)
                rc = rpool.tile([128, 512], F32, tag="rc")
                nc.vector.reciprocal(out=rc, in_=ps)
                recips.append(rc)

            # ---- AV -> h2 (normalized, fp8, carries the 16x of v) ----
            h28 = h28p.tile([128, CT, N], F8, tag="h28")
            for ct_ in range(CT):
                csl = slice(128 * ct_, 128 * (ct_ + 1))
                for nch in range(NCH):
                    qsl = slice(512 * nch, 512 * (nch + 1))
                    ps = psmain.tile([128, 512], F32, tag="ps")
                    for i in range(NT // 2):
                        nc.tensor.matmul(ps, lhsT=v8[:, 2*i:2*i+2, csl],
                                         rhs=a8[:, 2*i:2*i+2, qsl],
                                         start=(i == 0), stop=(i == NT // 2 - 1),
                                         perf_mode=DR)
                    nc.vector.tensor_tensor(h28[:, ct_, qsl], ps, recips[nch],
                                            AluOpType.mult)

            # ---- proj + residual + chunked store (1/256 undoes 16x*16x) ----
            for dt_ in range(CT):
                dsl = slice(128 * dt_, 128 * (dt_ + 1))
                for nch in range(NCH):
                    qsl = slice(512 * nch, 512 * (nch + 1))
                    ps = psmain.tile([128, 512], F32, tag="ps")
                    for i in range(CT // 2):
                        nc.tensor.matmul(ps, lhsT=wp_sb[:, 2*i:2*i+2, dsl],
                                         rhs=h28[:, 2*i:2*i+2, qsl],
                                         start=(i == 0), stop=(i == CT // 2 - 1),
                                         perf_mode=DR)
                    if pb_sb is not None:
                        tproj = scr.tile([128, N], F32, tag="scr")
                        nc.vector.tensor_scalar(tproj[:, qsl], ps,
                                                1.0 / (WSCALE * WSCALE), pb_sb[dt_],
                                                op0=AluOpType.mult, op1=AluOpType.add)
                        nc.vector.tensor_tensor(xt[dt_][:, qsl], tproj[:, qsl],
                                                xt[dt_][:, qsl], AluOpType.add)
                    else:
                        nc.vector.scalar_tensor_tensor(
                            out=xt[dt_][:, qsl], in0=ps,
                            scalar=1.0 / (WSCALE * WSCALE),
                            in1=xt[dt_][:, qsl],
                            op0=AluOpType.mult, op1=AluOpType.add)
                    nc.sync.dma_start(out=out_d[b, dt_, :, qsl],
                                      in_=xt[dt_][:, qsl])

        def body():
            # pipelined emission: batch b+1's x-load + GN are queued between
            # batch b's t/v projections and its attention, so GN(b+1)'s
            # DVE/ACT work hides under batch b's long PE stretch
            xt0 = xts.pop(0)
            h80 = group_norm(xt0)
            state = [(xt0, h80)]
            for b in range(1, BPC):
                t8, v8 = emit_tv(b - 1, *state[b - 1])
                if b not in xts:
                    xts[b] = load_x(b)
                xtb = xts.pop(b)
                state.append((xtb, group_norm(xtb)))
                emit_attn(b - 1, *state[b - 1], t8, v8)
            t8, v8 = emit_tv(BPC - 1, *state[BPC - 1])
            emit_attn(BPC - 1, *state[BPC - 1], t8, v8)

        if loop_reps is None:
            body()
        else:
            with tc.For_i(0, loop_reps, 1):
                body()
                xts[0] = load_x(0)


def _prep_inputs(x, gn_w, gn_b, q_w, q_b, k_w, k_b, v_w, v_b, p_w, p_b):
    f = np.float32
    f8 = mybir.dt.np(F8)

    def pack_w(w64):
        # w64: [D, C] output-major weight; stationary layout [128, CT, D]
        # stacked so [:, 2i:2i+2, :] is a DoubleRow [Ki, Ko=2, dim] pair
        wT = np.asarray(w64).T * WSCALE                       # [C, D]
        arr = wT.reshape(CT, 128, wT.shape[1]).transpose(1, 0, 2)
        return np.ascontiguousarray(np.clip(arr, -240.0, 240.0)).astype(f8)

    x = np.ascontiguousarray(np.asarray(x, f)).reshape(B, CT, 128, N)
    q64 = np.asarray(q_w, np.float64)
    k64 = np.asarray(k_w, np.float64)
    wm = k64.T @ q64                                          # scores = h^T wm h
    base = {
        "gmat": np.ascontiguousarray(
            np.kron(np.eye(128 // GS, dtype=f), np.ones((GS, GS), f)) / (GS * N)),
        "gnw": np.ascontiguousarray(np.asarray(gn_w, f).reshape(CT, 128).T),
        "gnb": np.ascontiguousarray(np.asarray(gn_b, f).reshape(CT, 128).T),
        "wm": pack_w(wm),
        "wv": pack_w(np.asarray(v_w, np.float64)),
        "wp": pack_w(np.asarray(p_w, np.float64)),
        "ones": np.ones((128, 2, 128), f).astype(f8),
    }
    qb_nz = bool(np.any(np.asarray(q_b)))
    vb_nz = bool(np.any(np.asarray(v_b)))
    pb_nz = bool(np.any(np.asarray(p_b)))
    if qb_nz:
        # after softmax, only the key-dependent score term matters:
        # s[key,q] += (q_b^T Wk) h[:,key];  uq = Wk^T q_b, scaled like weights
        uq = (k64.T @ np.asarray(q_b, np.float64)) * WSCALE
        base["uq"] = np.ascontiguousarray(
            np.clip(uq.reshape(CT, 128, 1).transpose(1, 0, 2), -240.0, 240.0)
        ).astype(f8)
    if vb_nz:
        base["vb"] = np.ascontiguousarray(
            np.broadcast_to(np.asarray(v_b, f)[None, :] * WSCALE, (128, C)).copy())
    if pb_nz:
        base["pb"] = np.ascontiguousarray(np.asarray(p_b, f)).reshape(CT, 128, 1)
    return x, base, (qb_nz, vb_nz, pb_nz)


def kernel(x, temb, gn_w, gn_b, q_w, q_b, k_w, k_b, v_w, v_b, p_w, p_b):
    global LAST_RESULTS
    del temb  # unused by the reference module
    assert not np.any(np.asarray(k_b)), "k bias folds out only when zero"
    x_r, base, flags = _prep_inputs(x, gn_w, gn_b, q_w, q_b, k_w, k_b,
                                    v_w, v_b, p_w, p_b)
    if flags not in _PROGRAM_CACHE:
        _PROGRAM_CACHE[flags] = _build_program(flags)
    nc = _PROGRAM_CACHE[flags]

    in_maps = [dict(base, xs=np.ascontiguousarray(x_r[BPC * i: BPC * (i + 1)]))
               for i in range(NCORES)]
    res = run_bass_kernel_spmd(nc, in_maps, core_ids=list(range(NCORES)))
    LAST_RESULTS = res
    out = np.concatenate([r["out"] for r in res.results], axis=0)
    return np.ascontiguousarray(out.reshape(B, C, H, W).astype(np.float32))
